# revision 2
# baseline (speedup 1.0000x reference)
"""Multi-head attention (B=8, N=1024, D=768, H=12) on 8 TRN2 NeuronCores.

Sharding: data-parallel over batch B — one batch element per core, weights
replicated, no collectives.

Per-core layout strategy (everything feature-major so no on-chip transposes):
  x^T [768, 1024] (host-transposed, bf16)
  Q/K feature-major [c, n]: lhsT = w_qkv block, rhs = x^T          -> QK_fm
  V token-major  [n, c]:    lhsT = x^T block,  rhs = w_qkv V cols  -> V_tm
  S^T[k, q] per (head, ktile): lhsT = K_fm [64,128], rhs = Q_fm [64,512]
     (two heads of a pair run as concurrent row-tiled matmuls: partitions
      0-63 / 64-127 -> tile_position (0,0)/(64,0))
  P^T = exp(SCALE * S^T)  on ACT, bf16 out
  AV^T + softmax-denominator in one matmul: lhsT = [V | ones] [128, 65]
     -> psum [65, q]: rows 0-63 = (P V)^T, row 64 = rowsum(P)
  normalize: r = 1/s via DVE, broadcast r along partitions via step-0
     DMA from DRAM, multiply on DVE -> A_fm bf16 (via DRAM rebase to
     [128, n] c_in-block tiles)
  proj: lhsT = w_proj block, rhs = A_fm -> out_fm [768, 1024] fp32 + bias
Host gathers out_fm per core and transposes back to [B, 1024, 768].
"""

import numpy as np
import ml_dtypes

import concourse.bass as bass
import concourse.tile as tile
from concourse import bacc, mybir

FP32 = mybir.dt.float32
BF16 = mybir.dt.bfloat16

B, N, D = 8, 1024, 768
H, HD = 12, 64
SCALE = float(HD) ** -0.5  # 0.125
CB = D // 128  # 6 contraction blocks of 128
PAIRS = H // 2  # 6 head pairs
KT = N // 128  # 8 key-token tiles
QB = N // 512  # 2 q blocks of 512
NCORES = 8


def build_attention(tc, outs, ins):
    from contextlib import ExitStack

    nc = tc.nc
    xT = ins["xT"]  # [768, 1024] bf16 dram
    wqkv = ins["w_qkv"]  # [768, 2304] bf16 dram
    wproj = ins["w_proj"]  # [768, 768] bf16 dram
    bproj = ins["b_proj"]  # [768] fp32 dram
    out = outs["out"]  # [768, 1024] fp32 dram

    Exp = mybir.ActivationFunctionType.Exp

    with ExitStack() as ctx:
        ec = ctx.enter_context
        sb_x = ec(tc.tile_pool(name="sb_x", bufs=CB))
        sb_wqkv = ec(tc.tile_pool(name="sb_wqkv", bufs=CB))
        sb_wproj = ec(tc.tile_pool(name="sb_wproj", bufs=CB))
        sb_bias = ec(tc.tile_pool(name="sb_bias", bufs=1))
        sb_qk = ec(tc.tile_pool(name="sb_qk", bufs=2 * PAIRS))
        sb_v = ec(tc.tile_pool(name="sb_v", bufs=KT))
        sb_pt = ec(tc.tile_pool(name="sb_pt", bufs=18))
        sb_stage = ec(tc.tile_pool(name="sb_stage", bufs=3))
        sb_rbc = ec(tc.tile_pool(name="sb_rbc", bufs=2))
        sb_anorm = ec(tc.tile_pool(name="sb_anorm", bufs=2))
        sb_attn = ec(tc.tile_pool(name="sb_attn", bufs=CB))
        sb_out = ec(tc.tile_pool(name="sb_out", bufs=2))
        sb_small = ec(tc.tile_pool(name="sb_small", bufs=4))
        ps_big = ec(tc.tile_pool(name="ps_big", bufs=3, space="PSUM"))
        ps_av = ec(tc.tile_pool(name="ps_av", bufs=2, space="PSUM"))
        dram = ec(tc.tile_pool(name="dram", bufs=1, space="DRAM"))

        # ---------- load x^T, weights, bias ----------
        x_sb = []
        for c in range(CB):
            xt = sb_x.tile([128, N], BF16, name=f"x{c}", tag="x")
            nc.sync.dma_start(xt, xT[c * 128 : (c + 1) * 128, :])
            x_sb.append(xt)
        wq_sb = []
        for c in range(CB):
            wt = sb_wqkv.tile([128, 3 * D], BF16, name=f"wqkv{c}", tag="wqkv")
            nc.sync.dma_start(wt, wqkv[c * 128 : (c + 1) * 128, :])
            wq_sb.append(wt)
        wp_sb = []
        for c in range(CB):
            wt = sb_wproj.tile([128, D], BF16, name=f"wp{c}", tag="wp")
            nc.sync.dma_start(wt, wproj[c * 128 : (c + 1) * 128, :])
            wp_sb.append(wt)
        bias_sb = sb_bias.tile([128, CB], FP32, name="bias")
        nc.sync.dma_start(bias_sb, bproj.rearrange("(a p) -> p a", p=128))

        s_dram = dram.tile([H, N], FP32, name="s_dram")
        r_dram = dram.tile([H, N], FP32, name="r_dram")
        a_dram = dram.tile([D, N], BF16, name="a_dram")

        # ---------- V token-major (+ ones column per head) ----------
        v_sb = []
        for t in range(KT):
            vps = ps_big.tile([128, D], FP32, name=f"vps{t}", tag="ps")
            for n0, nw in ((0, 512), (512, 256)):
                for c in range(CB):
                    nc.tensor.matmul(
                        vps[:, n0 : n0 + nw],
                        lhsT=x_sb[c][:, t * 128 : (t + 1) * 128],
                        rhs=wq_sb[c][:, 2 * D + n0 : 2 * D + n0 + nw],
                        start=(c == 0),
                        stop=(c == CB - 1),
                    )
            vt = sb_v.tile([128, H * 65], BF16, name=f"v{t}", tag="v")
            nc.vector.memset(vt, 1.0)
            nc.vector.tensor_copy(
                vt.rearrange("p (h e) -> p h e", h=H)[:, :, 0:HD],
                vps.rearrange("p (h e) -> p h e", h=H),
            )
            v_sb.append(vt)

        # ---------- per head pair: QK_fm, S^T, exp, AV, normalize ----------
        attn_sb = []
        for p in range(PAIRS):
            qk_t = []
            for which in (0, 1):  # 0 = Q, 1 = K
                col0 = which * D + p * 128
                ps = ps_big.tile([128, N], FP32, name=f"qkps{which}_{p}", tag="ps")
                for qb in range(QB):
                    for c in range(CB):
                        nc.tensor.matmul(
                            ps[:, qb * 512 : (qb + 1) * 512],
                            lhsT=wq_sb[c][:, col0 : col0 + 128],
                            rhs=x_sb[c][:, qb * 512 : (qb + 1) * 512],
                            start=(c == 0),
                            stop=(c == CB - 1),
                        )
                qkt = sb_qk.tile([128, N], BF16, name=f"qk{which}_{p}", tag="qk")
                nc.vector.tensor_copy(qkt, ps)
                qk_t.append(qkt)
            q_t, k_t = qk_t

            # S^T and exp, heads interleaved for PE row-group concurrency
            pt_tiles = {}
            for kt in range(KT):
                for j in (0, 1):
                    h = 2 * p + j
                    st = ps_big.tile([128, N], FP32, name=f"st{h}_{kt}", tag="ps")
                    for qb in range(QB):
                        nc.tensor.matmul(
                            st[:, qb * 512 : (qb + 1) * 512],
                            lhsT=k_t[j * 64 : (j + 1) * 64, kt * 128 : (kt + 1) * 128],
                            rhs=q_t[j * 64 : (j + 1) * 64, qb * 512 : (qb + 1) * 512],
                            start=True,
                            stop=True,
                        )
                    pt = sb_pt.tile([128, N], BF16, name=f"pt{h}_{kt}", tag="pt")
                    nc.scalar.activation(pt, st, Exp, scale=SCALE)
                    pt_tiles[(j, kt)] = pt

            # AV (+ row sums in partition 64)
            stage_tiles = []
            for j in (0, 1):
                h = 2 * p + j
                stage = sb_stage.tile([65, N], FP32, name=f"stage{h}", tag="stage")
                for qb in range(QB):
                    av = ps_av.tile([65, 512], FP32, name=f"av{h}_{qb}", tag="av")
                    for kt in range(KT):
                        nc.tensor.matmul(
                            av,
                            lhsT=v_sb[kt][:, h * 65 : (h + 1) * 65],
                            rhs=pt_tiles[(j, kt)][:, qb * 512 : (qb + 1) * 512],
                            start=(kt == 0),
                            stop=(kt == KT - 1),
                        )
                    nc.vector.tensor_copy(stage[:, qb * 512 : (qb + 1) * 512], av)
                nc.sync.dma_start(s_dram[h : h + 1, :], stage[64:65, :])
                stage_tiles.append(stage)

            # softmax denominators -> reciprocal -> partition-broadcast
            s_sb = sb_small.tile([2, N], FP32, name=f"ssb{p}", tag="sm")
            nc.sync.dma_start(s_sb, s_dram[2 * p : 2 * p + 2, :])
            r_sb = sb_small.tile([2, N], FP32, name=f"rsb{p}", tag="sm")
            nc.vector.reciprocal_approx_fast(r_sb, s_sb)
            nc.sync.dma_start(r_dram[2 * p : 2 * p + 2, :], r_sb)
            for j in (0, 1):
                h = 2 * p + j
                rbc = sb_rbc.tile([64, N], FP32, name=f"rbc{h}", tag="rbc")
                src = r_dram[h : h + 1, :]
                bcast = bass.AP(
                    tensor=src.tensor, offset=src.offset, ap=[[0, 64]] + src.ap[-1:]
                )
                nc.gpsimd.dma_start(rbc, bcast)
                an = sb_anorm.tile([64, N], BF16, name=f"an{h}", tag="an")
                nc.vector.tensor_mul(an, stage_tiles[j][0:64, :], rbc)
                nc.sync.dma_start(a_dram[h * 64 : (h + 1) * 64, :], an)
            at = sb_attn.tile([128, N], BF16, name=f"attn{p}", tag="attn")
            nc.sync.dma_start(at, a_dram[p * 128 : (p + 1) * 128, :])
            attn_sb.append(at)

        # ---------- output projection + bias ----------
        for mb in range(CB):
            ps = ps_big.tile([128, N], FP32, name=f"projps{mb}", tag="ps")
            for qb in range(QB):
                for c in range(CB):
                    nc.tensor.matmul(
                        ps[:, qb * 512 : (qb + 1) * 512],
                        lhsT=wp_sb[c][:, mb * 128 : (mb + 1) * 128],
                        rhs=attn_sb[c][:, qb * 512 : (qb + 1) * 512],
                        start=(c == 0),
                        stop=(c == CB - 1),
                    )
            ot = sb_out.tile([128, N], FP32, name=f"out{mb}", tag="out")
            nc.vector.tensor_scalar_add(ot, ps, bias_sb[:, mb : mb + 1])
            nc.sync.dma_start(out[mb * 128 : (mb + 1) * 128, :], ot)


def build_nc():
    nc = bacc.Bacc(
        "TRN2", target_bir_lowering=False, debug=False, num_devices=NCORES
    )
    ins = {
        "xT": nc.dram_tensor("xT", [D, N], BF16, kind="ExternalInput").ap(),
        "w_qkv": nc.dram_tensor("w_qkv", [D, 3 * D], BF16, kind="ExternalInput").ap(),
        "w_proj": nc.dram_tensor("w_proj", [D, D], BF16, kind="ExternalInput").ap(),
        "b_proj": nc.dram_tensor("b_proj", [D], FP32, kind="ExternalInput").ap(),
    }
    outs = {"out": nc.dram_tensor("out", [D, N], FP32, kind="ExternalOutput").ap()}
    with tile.TileContext(nc) as tc:
        build_attention(tc, outs, ins)
    nc.compile()
    return nc


def make_in_maps(x, w_qkv, w_proj, b_proj):
    xT = np.ascontiguousarray(
        np.transpose(np.asarray(x, np.float32), (0, 2, 1))
    ).astype(ml_dtypes.bfloat16)
    wq = np.asarray(w_qkv, np.float32).astype(ml_dtypes.bfloat16)
    wp = np.asarray(w_proj, np.float32).astype(ml_dtypes.bfloat16)
    bp = np.ascontiguousarray(np.asarray(b_proj, np.float32))
    return [
        {"xT": np.ascontiguousarray(xT[b]), "w_qkv": wq, "w_proj": wp, "b_proj": bp}
        for b in range(B)
    ]


_BUILT = None


def _get_built():
    global _BUILT
    if _BUILT is None:
        _BUILT = build_nc()
    return _BUILT


def kernel(x, w_qkv, w_proj, b_proj):
    from concourse.bass_utils import run_bass_kernel_spmd

    nc = _get_built()
    in_maps = make_in_maps(x, w_qkv, w_proj, b_proj)
    res = run_bass_kernel_spmd(nc, in_maps, core_ids=list(range(NCORES)))
    return np.stack(
        [np.asarray(res.results[b]["out"], np.float32).T for b in range(B)]
    )


# revision 3
# speedup vs baseline: 1.1312x; 1.1312x over previous
"""Multi-head attention (B=8, N=1024, D=768, H=12) on 8 TRN2 NeuronCores.

Sharding: data-parallel over batch B — one batch element per core, weights
replicated, no collectives.

Per-core layout strategy (everything feature-major so no on-chip transposes):
  x^T [768, 1024] (host-transposed, bf16)
  Q/K feature-major [c, n]: lhsT = w_qkv block, rhs = x^T          -> QK_fm
  V token-major  [n, c]:    lhsT = x^T block,  rhs = w_qkv V cols  -> V_tm
  S^T[k, q] per (head, ktile): lhsT = K_fm [64,128], rhs = Q_fm [64,512]
     (two heads of a pair run as concurrent row-tiled matmuls: partitions
      0-63 / 64-127 -> tile_position (0,0)/(64,0))
  P^T = exp(SCALE * S^T)  on ACT, bf16 out
  AV^T + softmax-denominator in one matmul: lhsT = [V | ones] [128, 65]
     -> psum [65, q]: rows 0-63 = (P V)^T, row 64 = rowsum(P)
  normalize: broadcast s along partitions via step-0 DMA through DRAM,
     r = 1/s via DVE reciprocal_approx_fast, multiply writes straight into
     the pair-packed proj input tile (cross-partition-base DVE write)
  proj: lhsT = w_proj block, rhs = A_fm -> out_fm [768, 1024] fp32 + bias
Host gathers out_fm per core and transposes back to [B, 1024, 768].

Emission order keeps the PE dense across head-pair boundaries (QK of pair
p+2 is emitted between S^T/exp and AV of pair p) so the HAM clock gate
stays at 8/8.
"""

import numpy as np
import ml_dtypes

import concourse.bass as bass
import concourse.tile as tile
from concourse import bacc, mybir

FP32 = mybir.dt.float32
BF16 = mybir.dt.bfloat16

B, N, D = 8, 1024, 768
H, HD = 12, 64
SCALE = float(HD) ** -0.5  # 0.125
CB = D // 128  # 6 contraction blocks of 128
PAIRS = H // 2  # 6 head pairs
KT = N // 128  # 8 key-token tiles
QB = N // 512  # 2 q blocks of 512
NCORES = 8


def build_attention(tc, outs, ins):
    from contextlib import ExitStack

    nc = tc.nc
    xT = ins["xT"]  # [768, 1024] bf16 dram
    wqkv = ins["w_qkv"]  # [768, 2304] bf16 dram
    wproj = ins["w_proj"]  # [768, 768] bf16 dram
    bproj = ins["b_proj"]  # [768] fp32 dram
    out = outs["out"]  # [768, 1024] fp32 dram

    Exp = mybir.ActivationFunctionType.Exp

    with ExitStack() as ctx:
        ec = ctx.enter_context
        sb_x = ec(tc.tile_pool(name="sb_x", bufs=CB))
        sb_wqk = ec(tc.tile_pool(name="sb_wqk", bufs=CB))
        sb_wv = ec(tc.tile_pool(name="sb_wv", bufs=CB))
        sb_wproj = ec(tc.tile_pool(name="sb_wproj", bufs=CB))
        sb_bias = ec(tc.tile_pool(name="sb_bias", bufs=1))
        sb_qk = ec(tc.tile_pool(name="sb_qk", bufs=2 * PAIRS))
        sb_v = ec(tc.tile_pool(name="sb_v", bufs=KT))
        sb_pt = ec(tc.tile_pool(name="sb_pt", bufs=18))
        sb_stage = ec(tc.tile_pool(name="sb_stage", bufs=4))
        sb_sbc = ec(tc.tile_pool(name="sb_sbc", bufs=2))
        sb_rbc = ec(tc.tile_pool(name="sb_rbc", bufs=2))
        sb_attn = ec(tc.tile_pool(name="sb_attn", bufs=CB))
        sb_out = ec(tc.tile_pool(name="sb_out", bufs=2))
        ps_big = ec(tc.tile_pool(name="ps_big", bufs=3, space="PSUM"))
        ps_av = ec(tc.tile_pool(name="ps_av", bufs=2, space="PSUM"))
        dram = ec(tc.tile_pool(name="dram", bufs=1, space="DRAM"))

        # ---------- x^T and Q/K weight columns first (PE starts early) ----
        x_sb = []
        for c in range(CB):
            xt = sb_x.tile([128, N], BF16, name=f"x{c}", tag="x")
            nc.sync.dma_start(xt, xT[c * 128 : (c + 1) * 128, :])
            x_sb.append(xt)
        wqk_sb = []
        for c in range(CB):
            wt = sb_wqk.tile([128, 2 * D], BF16, name=f"wqk{c}", tag="wqk")
            nc.sync.dma_start(wt, wqkv[c * 128 : (c + 1) * 128, 0 : 2 * D])
            wqk_sb.append(wt)
        bias_sb = sb_bias.tile([128, CB], FP32, name="bias")
        nc.sync.dma_start(bias_sb, bproj.rearrange("(a p) -> p a", p=128))
        s_dram = dram.tile([H, N], FP32, name="s_dram")

        qk_sb = {}  # (which, pair) -> [128, N] bf16

        def emit_qk(p):
            for which in (0, 1):  # 0 = Q, 1 = K
                col0 = which * D + p * 128
                ps = ps_big.tile([128, N], FP32, name=f"qkps{which}_{p}", tag="ps")
                for qb in range(QB):
                    for c in range(CB):
                        nc.tensor.matmul(
                            ps[:, qb * 512 : (qb + 1) * 512],
                            lhsT=wqk_sb[c][:, col0 : col0 + 128],
                            rhs=x_sb[c][:, qb * 512 : (qb + 1) * 512],
                            start=(c == 0),
                            stop=(c == CB - 1),
                        )
                qkt = sb_qk.tile([128, N], BF16, name=f"qk{which}_{p}", tag="qk")
                nc.vector.tensor_copy(qkt, ps)
                qk_sb[(which, p)] = qkt

        emit_qk(0)

        # ---------- V weights + V token-major (+ ones column per head) ----
        wv_sb = []
        for c in range(CB):
            wt = sb_wv.tile([128, D], BF16, name=f"wv{c}", tag="wv")
            nc.sync.dma_start(wt, wqkv[c * 128 : (c + 1) * 128, 2 * D : 3 * D])
            wv_sb.append(wt)
        wp_sb = []
        for c in range(CB):
            wt = sb_wproj.tile([128, D], BF16, name=f"wp{c}", tag="wp")
            nc.sync.dma_start(wt, wproj[c * 128 : (c + 1) * 128, :])
            wp_sb.append(wt)

        v_sb = []
        for t in range(KT):
            vps = ps_big.tile([128, D], FP32, name=f"vps{t}", tag="ps")
            for n0, nw in ((0, 512), (512, 256)):
                for c in range(CB):
                    nc.tensor.matmul(
                        vps[:, n0 : n0 + nw],
                        lhsT=x_sb[c][:, t * 128 : (t + 1) * 128],
                        rhs=wv_sb[c][:, n0 : n0 + nw],
                        start=(c == 0),
                        stop=(c == CB - 1),
                    )
            vt = sb_v.tile([128, H * 65], BF16, name=f"v{t}", tag="v")
            nc.vector.memset(vt, 1.0)
            nc.vector.tensor_copy(
                vt.rearrange("p (h e) -> p h e", h=H)[:, :, 0:HD],
                vps.rearrange("p (h e) -> p h e", h=H),
            )
            v_sb.append(vt)

        emit_qk(1)

        # ---------- per head pair: S^T, exp, AV, normalize ----------
        attn_sb = []
        for p in range(PAIRS):
            q_t, k_t = qk_sb[(0, p)], qk_sb[(1, p)]

            # S^T and exp, heads interleaved for PE row-group concurrency
            pt_tiles = {}
            for kt in range(KT):
                for j in (0, 1):
                    h = 2 * p + j
                    st = ps_big.tile([128, N], FP32, name=f"st{h}_{kt}", tag="ps")
                    for qb in range(QB):
                        nc.tensor.matmul(
                            st[:, qb * 512 : (qb + 1) * 512],
                            lhsT=k_t[j * 64 : (j + 1) * 64, kt * 128 : (kt + 1) * 128],
                            rhs=q_t[j * 64 : (j + 1) * 64, qb * 512 : (qb + 1) * 512],
                            start=True,
                            stop=True,
                        )
                    pt = sb_pt.tile([128, N], BF16, name=f"pt{h}_{kt}", tag="pt")
                    nc.scalar.activation(pt, st, Exp, scale=SCALE)
                    pt_tiles[(j, kt)] = pt

            # keep PE dense through the pair boundary
            if p + 2 < PAIRS:
                emit_qk(p + 2)

            # AV (+ row sums in partition 64), then normalize into the
            # pair-packed proj input tile
            at = sb_attn.tile([128, N], BF16, name=f"attn{p}", tag="attn")
            attn_sb.append(at)
            for j in (0, 1):
                h = 2 * p + j
                stage = sb_stage.tile([65, N], FP32, name=f"stage{h}", tag="stage")
                for qb in range(QB):
                    av = ps_av.tile([65, 512], FP32, name=f"av{h}_{qb}", tag="av")
                    for kt in range(KT):
                        nc.tensor.matmul(
                            av,
                            lhsT=v_sb[kt][:, h * 65 : (h + 1) * 65],
                            rhs=pt_tiles[(j, kt)][:, qb * 512 : (qb + 1) * 512],
                            start=(kt == 0),
                            stop=(kt == KT - 1),
                        )
                    nc.vector.tensor_copy(stage[:, qb * 512 : (qb + 1) * 512], av)
                # denominator row -> DRAM -> partition-broadcast back
                nc.sync.dma_start(s_dram[h : h + 1, :], stage[64:65, :])
                sbc = sb_sbc.tile([64, N], FP32, name=f"sbc{h}", tag="sbc")
                src = s_dram[h : h + 1, :]
                bcast = bass.AP(
                    tensor=src.tensor, offset=src.offset, ap=[[0, 64]] + src.ap[-1:]
                )
                nc.gpsimd.dma_start(sbc, bcast)
                rbc = sb_rbc.tile([64, N], FP32, name=f"rbc{h}", tag="rbc")
                nc.vector.reciprocal_approx_fast(rbc, sbc)
                nc.vector.tensor_mul(at[j * 64 : (j + 1) * 64, :], stage[0:64, :], rbc)

        # ---------- output projection + bias ----------
        for mb in range(CB):
            ps = ps_big.tile([128, N], FP32, name=f"projps{mb}", tag="ps")
            for qb in range(QB):
                for c in range(CB):
                    nc.tensor.matmul(
                        ps[:, qb * 512 : (qb + 1) * 512],
                        lhsT=wp_sb[c][:, mb * 128 : (mb + 1) * 128],
                        rhs=attn_sb[c][:, qb * 512 : (qb + 1) * 512],
                        start=(c == 0),
                        stop=(c == CB - 1),
                    )
            ot = sb_out.tile([128, N], FP32, name=f"out{mb}", tag="out")
            nc.vector.tensor_scalar_add(ot, ps, bias_sb[:, mb : mb + 1])
            nc.sync.dma_start(out[mb * 128 : (mb + 1) * 128, :], ot)


def build_nc():
    nc = bacc.Bacc(
        "TRN2", target_bir_lowering=False, debug=False, num_devices=NCORES
    )
    ins = {
        "xT": nc.dram_tensor("xT", [D, N], BF16, kind="ExternalInput").ap(),
        "w_qkv": nc.dram_tensor("w_qkv", [D, 3 * D], BF16, kind="ExternalInput").ap(),
        "w_proj": nc.dram_tensor("w_proj", [D, D], BF16, kind="ExternalInput").ap(),
        "b_proj": nc.dram_tensor("b_proj", [D], FP32, kind="ExternalInput").ap(),
    }
    outs = {"out": nc.dram_tensor("out", [D, N], FP32, kind="ExternalOutput").ap()}
    with tile.TileContext(nc) as tc:
        build_attention(tc, outs, ins)
    nc.compile()
    return nc


def make_in_maps(x, w_qkv, w_proj, b_proj):
    xT = np.ascontiguousarray(
        np.transpose(np.asarray(x, np.float32), (0, 2, 1))
    ).astype(ml_dtypes.bfloat16)
    wq = np.asarray(w_qkv, np.float32).astype(ml_dtypes.bfloat16)
    wp = np.asarray(w_proj, np.float32).astype(ml_dtypes.bfloat16)
    bp = np.ascontiguousarray(np.asarray(b_proj, np.float32))
    return [
        {"xT": np.ascontiguousarray(xT[b]), "w_qkv": wq, "w_proj": wp, "b_proj": bp}
        for b in range(B)
    ]


_BUILT = None


def _get_built():
    global _BUILT
    if _BUILT is None:
        _BUILT = build_nc()
    return _BUILT


def kernel(x, w_qkv, w_proj, b_proj):
    from concourse.bass_utils import run_bass_kernel_spmd

    nc = _get_built()
    in_maps = make_in_maps(x, w_qkv, w_proj, b_proj)
    res = run_bass_kernel_spmd(nc, in_maps, core_ids=list(range(NCORES)))
    return np.stack(
        [np.asarray(res.results[b]["out"], np.float32).T for b in range(B)]
    )
